# revision 1
# baseline (speedup 1.0000x reference)
"""UR-LSTM forward kernel for Trainium2 (8 NeuronCores).

Strategy (sequence-parallel with warmup):
  The UR-LSTM state is strongly contractive (forget gates bounded away from
  1), so a chunk of the sequence can be computed exactly (to fp32 noise) by
  starting W steps earlier from zero state.  T=1024 is split into 16 chunks;
  each of the 8 cores runs 2 independent chains.  Every chain runs
  S = C + W steps; the first W steps of chunks 1..15 are discarded warmup.

  Per step, per chain (B=128 full batch on every core):
    gates[2048, 128] = sum_k WtileT[k].T @ state_chunk[k]   (PE, bf16)
      where the contraction is over [h(512); x_t(10); 1; 0-pad] = 5 K-chunks
      of 128.  Bias b and the UR-LSTM fb offsets are folded into the ones-row
      column, so PSUM holds (f+fb, r-fb, u, o) pre-activations directly.
    f/r/u/o land in 4 separate PSUM banks (one per gate type).
    Elementwise is split: ScalarE (sigmoid/tanh), VectorE and GpSimd
    (arithmetic), with fp32 cell state and bf16 h output.
    y_t = W_out @ h_t + b_out is fused as 5 extra tiny matmuls per step.

  Two chains per core pipeline: while the PE runs chain B's matmuls, the
  vector engines run chain A's elementwise chain.
"""

import numpy as np
import ml_dtypes

B, T, I, H = 128, 1024, 10, 512
G4 = 4 * H  # 2048
NCORES = 8
NCHUNK = 16
W_WARM = 32
C_OUT = (T - W_WARM) // NCHUNK  # 60
S_STEPS = C_OUT + W_WARM  # 124
KCH = 5  # 4 h-chunks + 1 (x | ones | pad) chunk
GT = 16  # gate tiles of 128

_cache = {}


def _build_nc(S):
    import concourse.bacc as bacc
    import concourse.mybir as mybir
    import concourse.tile as tile

    dt = mybir.dt
    f32, bf16 = dt.float32, dt.bfloat16
    AF = mybir.ActivationFunctionType
    OP = mybir.AluOpType

    nc = bacc.Bacc(None, target_bir_lowering=False)

    w_d = nc.dram_tensor("w", [128, KCH * GT * 128], bf16, kind="ExternalInput")
    wy_d = nc.dram_tensor("wy", [128, KCH * 10], bf16, kind="ExternalInput")
    x_d = [
        nc.dram_tensor(f"x{c}", [128, S * 128], bf16, kind="ExternalInput")
        for c in range(2)
    ]
    y_d = [
        nc.dram_tensor(f"y{c}", [S, 10, 128], f32, kind="ExternalOutput")
        for c in range(2)
    ]

    with tile.TileContext(nc) as tc:
        with (
            tc.tile_pool(name="const", bufs=1) as const,
            tc.tile_pool(name="hpool", bufs=2) as hpool,
            tc.tile_pool(name="ew", bufs=3) as ew,
            tc.tile_pool(name="gpsum", bufs=6, space="PSUM") as gpsum,
            tc.tile_pool(name="ypsum", bufs=2, space="PSUM") as ypsum,
            tc.tile_pool(name="yout", bufs=4) as youtp,
        ):
            wbuf = const.tile([128, KCH * GT * 128], bf16, tag="wbuf")
            nc.sync.dma_start(wbuf[:], w_d[:])
            wybuf = const.tile([128, KCH * 10], bf16, tag="wybuf")
            nc.sync.dma_start(wybuf[:], wy_d[:])
            xb = []
            for c in range(2):
                t = const.tile([128, S * 128], bf16, tag=f"xb{c}")
                nc.sync.dma_start(t[:], x_d[c][:])
                xb.append(t)

            cbuf = []
            h_prev = []
            for c in range(2):
                ct = const.tile([128, H], f32, tag=f"cbuf{c}")
                nc.vector.memset(ct[:], 0.0)
                cbuf.append(ct)
                ht = hpool.tile([128, H], bf16, tag=f"h{c}")
                nc.vector.memset(ht[:], 0.0)
                h_prev.append(ht)

            def rhs_chunk(c, s, k):
                if k < 4:
                    return h_prev[c][:, k * 128 : (k + 1) * 128]
                return xb[c][:, s * 128 : (s + 1) * 128]

            for s in range(S):
                for c in range(2):
                    # ---- gates matmuls: 4 banks (f, r, u, o) ----
                    banks = [
                        gpsum.tile([128, 512], f32, tag="gbank", name=f"gbank{i}")
                        for i in range(4)
                    ]
                    for gt in range(GT):
                        bank = banks[gt // 4]
                        col = (gt % 4) * 128
                        out = bank[:, col : col + 128]
                        for k in range(KCH):
                            nc.tensor.matmul(
                                out,
                                lhsT=wbuf[:, (k * GT + gt) * 128 : (k * GT + gt + 1) * 128],
                                rhs=rhs_chunk(c, s, k),
                                start=(k == 0),
                                stop=(k == KCH - 1),
                            )

                    # ---- elementwise ----
                    fg = ew.tile([128, 512], f32, tag="fg")
                    rg = ew.tile([128, 512], f32, tag="rg")
                    tu = ew.tile([128, 512], f32, tag="tu")
                    og = ew.tile([128, 512], f32, tag="og")
                    nc.scalar.activation(fg[:], banks[0][:], AF.Sigmoid)
                    nc.scalar.activation(rg[:], banks[1][:], AF.Sigmoid)
                    nc.scalar.activation(tu[:], banks[2][:], AF.Tanh)
                    nc.scalar.activation(og[:], banks[3][:], AF.Sigmoid)

                    p = ew.tile([128, 512], f32, tag="p")
                    m = ew.tile([128, 512], f32, tag="m")
                    e = ew.tile([128, 512], f32, tag="e")
                    g = ew.tile([128, 512], f32, tag="g")
                    nc.vector.tensor_tensor(p[:], fg[:], fg[:], OP.mult)
                    nc.vector.tensor_tensor(m[:], fg[:], p[:], OP.subtract)
                    nc.vector.tensor_tensor(e[:], rg[:], m[:], OP.mult)
                    nc.vector.scalar_tensor_tensor(
                        g[:], e[:], 2.0, p[:], OP.mult, OP.add
                    )

                    wv = ew.tile([128, 512], f32, tag="wv")
                    zv = ew.tile([128, 512], f32, tag="zv")
                    nc.gpsimd.tensor_tensor(wv[:], cbuf[c][:], tu[:], OP.subtract)
                    nc.gpsimd.tensor_tensor(zv[:], g[:], wv[:], OP.mult)
                    nc.gpsimd.tensor_tensor(cbuf[c][:], zv[:], tu[:], OP.add)

                    tc2 = ew.tile([128, 512], f32, tag="tc2")
                    nc.scalar.activation(tc2[:], cbuf[c][:], AF.Tanh)
                    h_new = hpool.tile([128, H], bf16, tag=f"h{c}")
                    nc.vector.tensor_tensor(h_new[:], og[:], tc2[:], OP.mult)

                    # ---- fused output projection for this step ----
                    yp = ypsum.tile([10, 128], f32, tag="yp")
                    for k in range(KCH):
                        rhs = (
                            h_new[:, k * 128 : (k + 1) * 128]
                            if k < 4
                            else xb[c][:, s * 128 : (s + 1) * 128]
                        )
                        nc.tensor.matmul(
                            yp[:],
                            lhsT=wybuf[:, k * 10 : (k + 1) * 10],
                            rhs=rhs,
                            start=(k == 0),
                            stop=(k == KCH - 1),
                        )
                    yo = youtp.tile([10, 128], f32, tag="yo")
                    nc.scalar.activation(yo[:], yp[:], AF.Copy)
                    nc.sync.dma_start(y_d[c][s], yo[:])

                    h_prev[c] = h_new

    nc.compile()
    return nc


def _prep(inputs):
    x = np.asarray(inputs["x"], np.float32)
    W_ih = np.asarray(inputs["W_ih"], np.float32)
    W_hh = np.asarray(inputs["W_hh"], np.float32)
    b = np.asarray(inputs["b"], np.float32)
    fb = np.asarray(inputs["fb"], np.float32)
    W_out = np.asarray(inputs["W_out"], np.float32)
    b_out = np.asarray(inputs["b_out"], np.float32)
    bf = ml_dtypes.bfloat16

    bias_col = b.copy()
    bias_col[0:H] += fb
    bias_col[H : 2 * H] -= fb

    extra = np.zeros((128, G4), np.float32)
    extra[0:I] = W_ih.T
    extra[I] = bias_col
    Wfull = np.concatenate([W_hh.T, extra], axis=0)  # [640, 2048]
    w_host = (
        Wfull.reshape(KCH, 128, GT, 128).transpose(1, 0, 2, 3).reshape(128, -1)
    ).astype(bf)

    extra_y = np.zeros((128, 10), np.float32)
    extra_y[I] = b_out
    Wyfull = np.concatenate([W_out.T, extra_y], axis=0)  # [640, 10]
    wy_host = Wyfull.reshape(KCH, 128, 10).transpose(1, 0, 2).reshape(128, -1).astype(bf)

    xc = []
    for j in range(NCHUNK):
        start = j * C_OUT
        xs = x[:, start : start + S_STEPS, :]  # [128, S, 10]
        arr = np.zeros((128, S_STEPS * 128), np.float32)
        arr[0:I] = xs.transpose(2, 1, 0).reshape(I, -1)
        arr[I] = 1.0
        xc.append(arr.astype(bf))
    return w_host, wy_host, xc


def kernel(**inputs):
    from concourse.bass_utils import run_bass_kernel_spmd

    if "nc" not in _cache:
        _cache["nc"] = _build_nc(S_STEPS)
    nc = _cache["nc"]

    w_host, wy_host, xc = _prep(inputs)
    in_maps = []
    for core in range(NCORES):
        in_maps.append(
            {
                "w": w_host,
                "wy": wy_host,
                "x0": xc[2 * core],
                "x1": xc[2 * core + 1],
            }
        )
    res = run_bass_kernel_spmd(nc, in_maps, list(range(NCORES))).results

    y = np.zeros((B, T, 10), np.float32)
    for j in range(NCHUNK):
        core, chain = j // 2, j % 2
        yj = np.asarray(res[core][f"y{chain}"], np.float32)  # [S, 10, 128]
        yj = yj.transpose(2, 0, 1)  # [B, S, 10]
        if j == 0:
            y[:, 0:S_STEPS, :] = yj
        else:
            start = j * C_OUT + W_WARM
            y[:, start : start + C_OUT, :] = yj[:, W_WARM:, :]
    return y



# revision 4
# speedup vs baseline: 1.5586x; 1.5586x over previous
"""UR-LSTM forward kernel for Trainium2 (8 NeuronCores), v2.

Sequence-parallel with warmup: T=1024 split into 16 chunks of C=64; each of
the 8 cores runs 2 chains (time chunks) in lockstep, interleaved as the
column halves of 256-wide matmuls so every weight-tile load is amortized
over both chains.  Each chain runs S = C + W steps; the first W=12 steps are
discarded warmup (UR-LSTM state is contractive; measured rel-err 7e-3).

Per superstep (both chains advance one t):
  gates[2048, 256] = sum_k Wtile[k].T @ [h_A | h_B]  in bf16, fp32 PSUM.
  16 gate tiles x 5 K-chunks (4 h-chunks + 1 x/bias chunk).  PSUM bank b
  holds gate tiles 2b, 2b+1 side by side; groups within a bank are strictly
  sequential (x start=True, then 4 h-chunk accumulates).
  Gate phase order f, r, u, o lets the g/c elementwise chain (all bf16 on
  DVE, 2x mode) hide under the u/o matmuls; sigmoid/tanh on ScalarE.
  Next-superstep x-matmuls + ring-buffered y-projection (batched, N=512)
  fill the PE tail while h(s) is finished.
"""

import numpy as np
import ml_dtypes

B, T, I, H = 128, 1024, 10, 512
NCORES = 8
W_WARM = 12
C_OUT = 64
S_STEPS = C_OUT + W_WARM  # 76
KCH = 5
GT = 16
RING = 8
NWIN = C_OUT // 4  # 16 y windows of 4 supersteps
XQ = 4  # x dma quarters
XQC = S_STEPS // XQ  # 19 supersteps per quarter

_cache = {}


def _build_nc():
    import concourse.bacc as bacc
    import concourse.mybir as mybir
    import concourse.tile as tile

    dt = mybir.dt
    f32, bf16 = dt.float32, dt.bfloat16
    AF = mybir.ActivationFunctionType
    OP = mybir.AluOpType

    nc = bacc.Bacc(None, target_bir_lowering=False)

    w_d = nc.dram_tensor("w", [128, KCH * GT * 128], bf16, kind="ExternalInput")
    wy_d = nc.dram_tensor("wy", [128, 4 * 10], bf16, kind="ExternalInput")
    bo_d = nc.dram_tensor("bout", [42, 1], f32, kind="ExternalInput")
    x_d = nc.dram_tensor("x", [128, S_STEPS * 256], bf16, kind="ExternalInput")
    y_d = nc.dram_tensor("y", [NWIN, 42, 512], f32, kind="ExternalOutput")

    with tile.TileContext(nc) as tc:
        with (
            tc.tile_pool(name="const", bufs=1) as const,
            tc.tile_pool(name="ew", bufs=2) as ew,
            tc.tile_pool(name="gpsum", bufs=7, space="PSUM") as gpsum,
            tc.tile_pool(name="ypsum", bufs=1, space="PSUM") as ypsum,
        ):
            wbuf = const.tile([128, KCH * GT * 128], bf16, tag="wbuf")
            nc.sync.dma_start(wbuf[:], w_d[:])
            wybuf = const.tile([128, 4 * 10], bf16, tag="wybuf")
            nc.sync.dma_start(wybuf[:], wy_d[:])
            bout = const.tile([42, 1], f32, tag="bout")
            nc.sync.dma_start(bout[:], bo_d[:])
            xq = []
            for q in range(XQ):
                t = const.tile([128, XQC * 256], bf16, tag=f"xq{q}")
                nc.sync.dma_start(t[:], x_d[:, q * XQC * 256:(q + 1) * XQC * 256])
                xq.append(t)

            hbuf = const.tile([128, RING, 4, 256], bf16, tag="hbuf")
            nc.vector.memset(hbuf[:], 0.0)
            cst = const.tile([128, 1024], bf16, tag="cst")
            nc.vector.memset(cst[:], 0.0)
            ybuf = const.tile([42, 1024], f32, tag="ybuf")

            def xrhs(s):
                q, r = divmod(s, XQC)
                return xq[q][:, r * 256:(r + 1) * 256]

            def wtile(k, t):
                return wbuf[:, (k * GT + t) * 128:(k * GT + t + 1) * 128]

            def xmm(gb, s, t):
                nc.tensor.matmul(
                    gb[t // 2][:, (t % 2) * 256:(t % 2) * 256 + 256],
                    lhsT=wtile(4, t), rhs=xrhs(s), start=True, stop=False)

            def ymms(w):
                # y projection for window w (supersteps W+4w .. W+4w+3)
                base = (W_WARM + 4 * w) % RING
                yp = ypsum.tile([42, 512], f32, tag="yp")
                for half, r0 in ((0, 0), (1, 32)):
                    for k in range(4):
                        nc.tensor.matmul(
                            yp[r0:r0 + 10, :],
                            lhsT=wybuf[:, k * 10:(k + 1) * 10],
                            rhs=hbuf[:, base:base + 4, k:k + 1,
                                     half * 128:half * 128 + 128],
                            start=(k == 0), stop=(k == 3))
                wc = (w % 2) * 512
                nc.scalar.add(ybuf[0:10, wc:wc + 512], yp[0:10, :], add=bout[0:10])
                nc.scalar.add(ybuf[32:42, wc:wc + 512], yp[32:42, :], add=bout[32:42])
                nc.sync.dma_start(y_d[w], ybuf[:, wc:wc + 512])

            gb_prev = None
            for s in range(S_STEPS):
                gb = [gpsum.tile([128, 512], f32, tag="gb", name=f"gb{s}_{i}")
                      for i in range(8)]
                prev = (s - 1) % RING
                # ---- tail block: x-MMs for this superstep + y for last window
                if s > 0:
                    for t in (0, 2, 4, 6, 8, 10):
                        xmm(gb, s, t)
                    if s > W_WARM and (s - W_WARM) % 4 == 0:
                        ymms((s - W_WARM) // 4 - 1)
                    xmm(gb, s, 12)
                    xmm(gb, s, 14)
                # ---- gate matmuls
                for t in range(16):
                    if (s == 0 and t % 2 == 0) or t % 2 == 1:
                        xmm(gb, s, t)
                    for k in range(4):
                        nc.tensor.matmul(
                            gb[t // 2][:, (t % 2) * 256:(t % 2) * 256 + 256],
                            lhsT=wtile(k, t),
                            rhs=hbuf[:, prev:prev + 1, k:k + 1, :],
                            start=False, stop=(k == 3))

                # ---- elementwise (bf16): halves hb cover gate dims hb*512..
                fg = ew.tile([128, 1024], bf16, tag="fg")
                rg = ew.tile([128, 1024], bf16, tag="rg")
                tug = ew.tile([128, 1024], bf16, tag="tug")
                og = ew.tile([128, 1024], bf16, tag="og")
                tch = ew.tile([128, 1024], bf16, tag="tch")
                p = ew.tile([128, 1024], bf16, tag="p")
                m = ew.tile([128, 1024], bf16, tag="m")
                e = ew.tile([128, 1024], bf16, tag="e")
                g = ew.tile([128, 1024], bf16, tag="g")
                wv = ew.tile([128, 1024], bf16, tag="wv")
                zv = ew.tile([128, 1024], bf16, tag="zv")

                def hv(x, hb):
                    return x[:, hb * 512:hb * 512 + 512]

                # ScalarE, in PE-completion order
                for hb in (0, 1):
                    nc.scalar.activation(hv(fg, hb), gb[0 + hb][:], AF.Sigmoid)
                for hb in (0, 1):
                    nc.scalar.activation(hv(rg, hb), gb[2 + hb][:], AF.Sigmoid)
                for hb in (0, 1):
                    nc.scalar.activation(hv(tug, hb), gb[4 + hb][:], AF.Tanh)
                for hb in (0, 1):
                    nc.scalar.activation(hv(og, hb), gb[6 + hb][:], AF.Sigmoid)

                # VectorE: g = 2*rg*(fg - fg^2) + fg^2 ; c = g*(c - tu) + tu
                for hb in (0, 1):
                    nc.vector.tensor_tensor(hv(p, hb), hv(fg, hb), hv(fg, hb), OP.mult)
                    nc.vector.tensor_tensor(hv(m, hb), hv(fg, hb), hv(p, hb), OP.subtract)
                for hb in (0, 1):
                    nc.vector.tensor_tensor(hv(e, hb), hv(rg, hb), hv(m, hb), OP.mult)
                    nc.vector.scalar_tensor_tensor(
                        hv(g, hb), hv(e, hb), 2.0, hv(p, hb), OP.mult, OP.add)
                    nc.vector.tensor_tensor(hv(wv, hb), hv(cst, hb), hv(tug, hb), OP.subtract)
                    nc.vector.tensor_tensor(hv(zv, hb), hv(g, hb), hv(wv, hb), OP.mult)
                    nc.vector.tensor_tensor(hv(cst, hb), hv(zv, hb), hv(tug, hb), OP.add)
                    nc.scalar.activation(hv(tch, hb), hv(cst, hb), AF.Tanh)
                # h = sigmoid(o) * tanh(c) -> ring slot s%RING
                slot = s % RING
                for hb in (0, 1):
                    nc.vector.tensor_tensor(
                        hbuf[:, slot:slot + 1, 2 * hb:2 * hb + 2, :],
                        hv(og, hb), hv(tch, hb), OP.mult)
                gb_prev = gb

            ymms(NWIN - 1)

    nc.compile()
    return nc


def _prep(inputs):
    x = np.asarray(inputs["x"], np.float32)
    W_ih = np.asarray(inputs["W_ih"], np.float32)
    W_hh = np.asarray(inputs["W_hh"], np.float32)
    b = np.asarray(inputs["b"], np.float32)
    fb = np.asarray(inputs["fb"], np.float32)
    W_out = np.asarray(inputs["W_out"], np.float32)
    b_out = np.asarray(inputs["b_out"], np.float32)
    bf = ml_dtypes.bfloat16

    bias_col = b.copy()
    bias_col[0:H] += fb
    bias_col[H:2 * H] -= fb

    extra = np.zeros((128, 4 * H), np.float32)
    extra[0:I] = W_ih.T
    extra[I] = bias_col
    Wfull = np.concatenate([W_hh.T, extra], axis=0)  # [640, 2048]
    w_host = (Wfull.reshape(KCH, 128, GT, 128).transpose(1, 0, 2, 3)
              .reshape(128, -1)).astype(bf)

    wy_host = (W_out.T.reshape(4, 128, 10).transpose(1, 0, 2)
               .reshape(128, -1)).astype(bf)
    bo_host = np.zeros((42, 1), np.float32)
    bo_host[0:10, 0] = b_out
    bo_host[32:42, 0] = b_out

    xc = []
    for core in range(NCORES):
        arr = np.zeros((128, S_STEPS, 2, 128), np.float32)
        for c in range(2):
            j = 2 * core + c
            t0 = j * C_OUT - W_WARM
            lo = max(0, -t0)  # first valid s
            ts = np.arange(t0 + lo, t0 + S_STEPS)
            arr[0:I, lo:, c, :] = x[:, ts, :].transpose(2, 1, 0)
            arr[I, lo:, c, :] = 1.0
        xc.append(arr.reshape(128, -1).astype(bf))
    return w_host, wy_host, bo_host, xc


def make_in_maps(inputs):
    w_host, wy_host, bo_host, xc = _prep(inputs)
    return [
        {"w": w_host, "wy": wy_host, "bout": bo_host, "x": xc[core]}
        for core in range(NCORES)
    ]


def kernel(**inputs):
    from concourse.bass_utils import run_bass_kernel_spmd

    if "nc" not in _cache:
        _cache["nc"] = _build_nc()
    nc = _cache["nc"]

    in_maps = make_in_maps(inputs)
    res = run_bass_kernel_spmd(nc, in_maps, list(range(NCORES))).results

    y = np.zeros((B, T, 10), np.float32)
    for core in range(NCORES):
        yc = np.asarray(res[core]["y"], np.float32)  # [NWIN, 42, 512]
        for c, r0 in ((0, 0), (1, 32)):
            j = 2 * core + c
            # [NWIN, 10, 4, 128] -> [128, NWIN*4, 10]
            yj = yc[:, r0:r0 + 10, :].reshape(NWIN, 10, 4, 128)
            y[:, j * C_OUT:(j + 1) * C_OUT, :] = yj.transpose(3, 0, 2, 1).reshape(
                128, C_OUT, 10)
    return y
